# revision 24
# baseline (speedup 1.0000x reference)
"""Trainium2 Bass kernel for nn_KANSplineLayer (KAN spline layer, 8-core SPMD).

Math rewrite (validated 2.9e-3 rel err vs reference on HW, tolerance 2e-2):
  reference: out = silu(BN_b(x @ Wb)) + BN_s(basis(minmax(x)) @ Ws.T)
  Spline is CPWL on t = 4*xn in [0,4] with kinks at {1,2,3}:
    out = silu(x@Wb + b_b) + t@Wt + relu(t-1)@H1 + relu(t-2)@H2 + relu(t-3)@H3 + C_s
  (contraction 4*256 instead of 9*256). All GEMM operands bf16 (fp32r streams
  at half PE rate; bf16 is full rate), fp32 PSUM accumulate.

Structure: TWO NEFF launches with a host-side 2KB-scale min/max combine
between them (the all-reduce across shards, host-mediated):
  NEFF-1: per-core DMA of its row shard (bf16) + running pairwise min/max on
      DVE -> per-core partial [128, 2, 256]. ~15us.
  host: reduce 8 x [128,2,256] partials -> global gmin/gmax per feature,
      s4 = 4/(range+eps), gs = gmin*s4 (f32, exact on bf16 values).
  NEFF-2: full compute with s4/gs as inputs. ~50us.
Measured alternatives that justify this: the on-device NCCL 2KB AllReduce
costs ~68us wall (fixed ~21+40us entry barrier + ~11us trigger delay + mesh
AR ~13-22us) and gates everything s4-dependent behind ~89us; a remote-DMA
XOR-butterfly (correct on HW) has ms-scale delivery latency in this axon
sandbox (host-emulated fabric), so neither beats two clean launches.

Other schedule decisions (from traces of earlier versions):
  * All GEMMs weight-stationary with TRANSPOSED output [out_f, rows]: one
    LDWEIGHTS per [128,128] weight block serves N=512-row matmuls
    (data-stationary pays LDW per matmul at ~275-300ns/MM: same row group ->
    no LDW/MM overlap). Out-feature biases (b_b inside silu, C_s in the
    epilogue) become per-partition scalars -- no rank-1 bias matmuls. Output
    is DMA'd transposed; the host untransposes (free).
  * Planes t,r1,r3 on DVE (tensor_scalar), r2 on ACT; ACT measured ~80-94
    G el/s so it also gets half the PSUM evacuations + silu, DVE the rest.
"""
import numpy as np
import ml_dtypes

import concourse.bacc as bacc
import concourse.bass as bass
import concourse.tile as tile
from concourse import mybir
from concourse.bass_utils import run_bass_kernel_spmd

# ---- problem constants (hardcoded; kernel.py must be self-contained) ----
IN_F, OUT_F = 256, 256
K_KNOTS = 9
EPS_MINMAX = 1e-7
EPS_BN = 1e-3
B, H, W = 32, 32, 32
N_TOTAL = B * H * W            # 32768 rows
N_CORES = 8
N_SHARD = N_TOTAL // N_CORES   # 4096 rows per core
R_TILES = N_SHARD // 128       # 32 row tiles per core
G_TILES = 4                    # row tiles per input DMA group
N_GROUPS = R_TILES // G_TILES  # 8 input DMA groups
CH = 512                       # chunk of rows (matmul moving N)
N_CHUNKS = N_SHARD // CH       # 8

F32 = mybir.dt.float32
BF16 = mybir.dt.bfloat16
NPBF16 = ml_dtypes.bfloat16


def _host_prep(base_weight, spline_weight, spline_scaler,
               bn_base_gamma, bn_base_beta, bn_base_mean, bn_base_var,
               bn_spline_gamma, bn_spline_beta, bn_spline_mean, bn_spline_var):
    """Fold BN + rewrite spline into relu-plane weights. All in float64."""
    f64 = np.float64
    w = np.asarray(spline_weight, f64) * np.asarray(spline_scaler, f64)[:, :, None]
    knots = np.linspace(-1.0, 1.0, K_KNOTS).astype(f64)
    jg = np.arange(5, dtype=f64) / 4.0
    tri = np.maximum(0.0, 1.0 - np.abs(jg[None, :] - knots[:, None]))   # [k, j]
    G = np.einsum('oik,kj->oij', w, tri)                                # [o,i,5]
    a_s = np.asarray(bn_spline_gamma, f64) / np.sqrt(np.asarray(bn_spline_var, f64) + EPS_BN)
    b_s = np.asarray(bn_spline_beta, f64) - a_s * np.asarray(bn_spline_mean, f64)
    G = G * a_s[:, None, None]
    W_t = (G[:, :, 1] - G[:, :, 0]).T                                   # [i,o]
    H1 = (G[:, :, 2] - 2 * G[:, :, 1] + G[:, :, 0]).T
    H2 = (G[:, :, 3] - 2 * G[:, :, 2] + G[:, :, 1]).T
    H3 = (G[:, :, 4] - 2 * G[:, :, 3] + G[:, :, 2]).T
    C_s = G[:, :, 0].sum(axis=1) + b_s                                  # [o]
    a_b = np.asarray(bn_base_gamma, f64) / np.sqrt(np.asarray(bn_base_var, f64) + EPS_BN)
    b_b = np.asarray(bn_base_beta, f64) - a_b * np.asarray(bn_base_mean, f64)
    Wb = np.asarray(base_weight, f64) * a_b[None, :]                    # [i,o]

    # weight blocks as lhsT [K=feat within fb, M=out within oh]
    planes = [W_t, H1, H2, H3]
    w_sp = np.empty((128, 4, 2, 2, 128), dtype=NPBF16)
    w_bs = np.empty((128, 2, 2, 128), dtype=NPBF16)
    for fb in range(2):
        for oh in range(2):
            for m, M in enumerate(planes):
                w_sp[:, m, fb, oh, :] = M[fb * 128:(fb + 1) * 128,
                                          oh * 128:(oh + 1) * 128].astype(NPBF16)
            w_bs[:, fb, oh, :] = Wb[fb * 128:(fb + 1) * 128,
                                    oh * 128:(oh + 1) * 128].astype(NPBF16)
    bbc = np.stack([b_b[0:128], b_b[128:256]], axis=1).astype(np.float32)  # [128,2]
    csc = np.stack([C_s[0:128], C_s[128:256]], axis=1).astype(np.float32)  # [128,2]
    return w_sp, w_bs, bbc, csc


def _build_minmax():
    """NEFF-1: per-shard running min/max + transpose x -> x^T writeback.

    DVE owns the min/max chain (the critical path); the otherwise-idle PE does
    the 64 [128,128] transposes and ACT + DVE split the PSUM evacuations.
    x^T goes back to DRAM so NEFF-2 starts matmuls immediately.
    """
    nc = bacc.Bacc(num_devices=N_CORES)
    x_sh = nc.declare_dram_parameter("x_sh", [N_SHARD, IN_F], BF16, isOutput=False)
    ident_d = nc.declare_dram_parameter("ident", [128, 128], BF16, isOutput=False)
    mm_out = nc.declare_dram_parameter("mm_out", [128, 2, 1024], BF16, isOutput=True)
    xt_d = nc.declare_dram_parameter("xt_sh", [2, 128, N_SHARD], BF16, isOutput=True)
    x_g = x_sh.rearrange("(g t p) f -> g p t f", g=N_GROUPS, t=G_TILES, p=128)
    xt_re = xt_d.rearrange("b p n -> p b n")

    from contextlib import ExitStack
    with tile.TileContext(nc) as tc, ExitStack() as es:
        cons = es.enter_context(tc.tile_pool(name="cons", bufs=1))
        stage = es.enter_context(tc.tile_pool(name="stage", bufs=2))
        xin_p = es.enter_context(tc.tile_pool(name="xin", bufs=3))
        xt_p = es.enter_context(tc.tile_pool(name="xtp", bufs=3))
        psT = es.enter_context(tc.tile_pool(name="psT", bufs=6, space="PSUM"))

        id_st = stage.tile([128, 128], BF16, tag="id_st")
        nc.sync.dma_start(out=id_st[:], in_=ident_d[:])
        ident = cons.tile([128, 128], BF16, tag="ident")
        nc.scalar.copy(out=ident[:], in_=id_st[:])

        accmin = cons.tile([128, G_TILES * IN_F], BF16, tag="accmin")
        accmax = cons.tile([128, G_TILES * IN_F], BF16, tag="accmax")
        for g in range(N_GROUPS):
            xin = xin_p.tile([128, G_TILES, IN_F], BF16, tag="xin", name=f"xin{g}")
            nc.sync.dma_start(out=xin[:], in_=x_g[g])
            v = xin[:].rearrange("p t f -> p (t f)")
            if g == 0:
                nc.vector.tensor_copy(out=accmin[:], in_=v)
                nc.vector.tensor_copy(out=accmax[:], in_=v)
            else:
                nc.vector.tensor_tensor(out=accmin[:], in0=accmin[:], in1=v,
                                        op=mybir.AluOpType.min)
                nc.vector.tensor_tensor(out=accmax[:], in0=accmax[:], in1=v,
                                        op=mybir.AluOpType.max)
            # transpose the group's 4 row tiles; evacs split DVE/ACT; DMA out
            xtg = xt_p.tile([128, 2, G_TILES * 128], BF16, tag="xtg",
                            name=f"xtg{g}")
            for t in range(G_TILES):
                for fb in range(2):
                    pst = psT.tile([128, 128], BF16, tag="pst")
                    nc.tensor.transpose(
                        pst[:], xin[:, t, fb * 128:(fb + 1) * 128], ident[:])
                    if (t * 2 + fb) % 8 < 3:
                        nc.vector.tensor_copy(
                            out=xtg[:, fb, t * 128:(t + 1) * 128], in_=pst[:])
                    else:
                        nc.scalar.copy(
                            out=xtg[:, fb, t * 128:(t + 1) * 128], in_=pst[:])
            nc.scalar.dma_start(
                out=xt_re[:, :, g * 512:(g + 1) * 512], in_=xtg[:])
        nc.gpsimd.dma_start(out=mm_out[:, 0, :], in_=accmin[:])
        nc.gpsimd.dma_start(out=mm_out[:, 1, :], in_=accmax[:])
    nc.compile()
    return nc


def _build_main():
    """NEFF-2: base path, planes, spline GEMMs, epilogue (reads x^T)."""
    nc = bacc.Bacc(num_devices=N_CORES)
    xt_d = nc.declare_dram_parameter("xt_sh", [2, 128, N_SHARD], BF16, isOutput=False)
    w_sp_d = nc.declare_dram_parameter("w_sp", [128, 4, 2, 2, 128], BF16, isOutput=False)
    w_bs_d = nc.declare_dram_parameter("w_bs", [128, 2, 2, 128], BF16, isOutput=False)
    bbc_d = nc.declare_dram_parameter("bbc", [128, 2], F32, isOutput=False)
    csc_d = nc.declare_dram_parameter("csc", [128, 2], F32, isOutput=False)
    s4c_d = nc.declare_dram_parameter("s4c", [128, 2], F32, isOutput=False)
    gsc_d = nc.declare_dram_parameter("gsc", [128, 2], F32, isOutput=False)
    out_sh = nc.declare_dram_parameter("out_sh", [2, 128, N_SHARD], F32, isOutput=True)
    xt_re = xt_d.rearrange("b p n -> p b n")

    from contextlib import ExitStack
    with tile.TileContext(nc) as tc, ExitStack() as es:
        cons = es.enter_context(tc.tile_pool(name="cons", bufs=1))
        stage = es.enter_context(tc.tile_pool(name="stage", bufs=2))
        planes_p = es.enter_context(tc.tile_pool(name="planes", bufs=2))
        outp = es.enter_context(tc.tile_pool(name="outp", bufs=4))
        psB = es.enter_context(tc.tile_pool(name="psB", bufs=2, space="PSUM"))
        psM = es.enter_context(tc.tile_pool(name="psM", bufs=2, space="PSUM"))

        xt = cons.tile([128, 2, N_SHARD], BF16, tag="xt")
        sb = cons.tile([128, 2, N_SHARD], BF16, tag="sb")

        def load(nm, shape, dram_ap, dt, eng):
            tmp = stage.tile(shape, dt, tag=f"st_{nm}", name=f"st_{nm}")
            nc.sync.dma_start(out=tmp[:], in_=dram_ap)
            dst = cons.tile(shape, dt, tag=nm, name=nm)
            if eng == "a":
                nc.scalar.copy(out=dst[:], in_=tmp[:])
            else:
                nc.vector.tensor_copy(out=dst[:], in_=tmp[:])
            return dst

        AF = mybir.ActivationFunctionType

        def emit_dma(c):
            csl = slice(c * CH, (c + 1) * CH)
            nc.sync.dma_start(out=xt[:, :, csl], in_=xt_re[:, :, csl])

        def emit_planes(c):
            csl = slice(c * CH, (c + 1) * CH)
            tpl = [[None, None] for _ in range(4)]
            for fb in range(2):
                t = planes_p.tile([128, CH], BF16, tag=f"t{fb}", name=f"t{fb}_{c}")
                nc.vector.tensor_scalar(
                    out=t[:], in0=xt[:, fb, csl],
                    scalar1=s4c[:, fb:fb + 1], scalar2=gsc[:, fb:fb + 1],
                    op0=mybir.AluOpType.mult, op1=mybir.AluOpType.subtract)
                tpl[0][fb] = t
                for m in (1, 2, 3):
                    rpl = planes_p.tile([128, CH], BF16, tag=f"r{m}{fb}",
                                        name=f"r{m}{fb}_{c}")
                    nc.vector.tensor_scalar(
                        out=rpl[:], in0=t[:], scalar1=float(m), scalar2=0.0,
                        op0=mybir.AluOpType.subtract, op1=mybir.AluOpType.max)
                    tpl[m][fb] = rpl
            return tpl

        def emit_base(c):
            csl = slice(c * CH, (c + 1) * CH)
            for oh in range(2):
                pb = psB.tile([128, CH], F32, tag=f"psb{oh}")
                nc.tensor.matmul(pb[:], w_bs[:, 0, oh, :], xt[:, 0, csl],
                                 start=True, stop=False, skip_group_check=True)
                nc.tensor.matmul(pb[:], w_bs[:, 1, oh, :], xt[:, 1, csl],
                                 start=False, stop=True, skip_group_check=True)
                nc.scalar.activation(
                    out=sb[:, oh, csl], in_=pb[:],
                    func=AF.Silu, bias=bbc[:, oh:oh + 1], scale=1.0)

        def emit_spline(c, tpl):
            csl = slice(c * CH, (c + 1) * CH)
            for oh in range(2):
                pm = psM.tile([128, CH], F32, tag=f"psm{oh}")
                first = True
                for m in range(4):
                    for fb in range(2):
                        nc.tensor.matmul(
                            pm[:], w_sp[:, m, fb, oh, :], tpl[m][fb][:],
                            start=first, stop=(m == 3 and fb == 1),
                            skip_group_check=True)
                        first = False
                o = outp.tile([128, CH], F32, tag=f"o{oh}", name=f"o{oh}_{c}")
                nc.vector.tensor_tensor(
                    out=o[:], in0=pm[:], in1=sb[:, oh, csl],
                    op=mybir.AluOpType.add)
                nc.vector.tensor_scalar(
                    out=o[:], in0=o[:], scalar1=csc[:, oh:oh + 1],
                    scalar2=None, op0=mybir.AluOpType.add)
                nc.sync.dma_start(out=out_sh[oh, :, csl], in_=o[:])

        w_bs = load("w_bs", [128, 2, 2, 128], w_bs_d[:], BF16, "a")
        bbc = load("bbc", [128, 2], bbc_d[:], F32, "a")
        s4c = load("s4c", [128, 2], s4c_d[:], F32, "v")
        gsc = load("gsc", [128, 2], gsc_d[:], F32, "v")
        csc = load("csc", [128, 2], csc_d[:], F32, "v")
        emit_dma(0)
        emit_dma(1)
        w_sp = load("w_sp", [128, 4, 2, 2, 128], w_sp_d[:], BF16, "a")

        tpl_prev = emit_planes(0)
        for c in range(N_CHUNKS):
            if c + 2 < N_CHUNKS:
                emit_dma(c + 2)
            emit_base(c)
            emit_spline(c, tpl_prev)
            if c + 1 < N_CHUNKS:
                tpl_prev = emit_planes(c + 1)
    nc.compile()
    return nc


_CACHE = {}


def _shard_x(inputs):
    xf = np.asarray(inputs["x"], np.float32).reshape(N_TOTAL, IN_F).astype(NPBF16)
    return [np.ascontiguousarray(xf[c * N_SHARD:(c + 1) * N_SHARD])
            for c in range(N_CORES)]


def host_combine_minmax(partials):
    """The cross-shard all-reduce of per-shard min/max partials (f32, exact on
    bf16 values), then s4 = 4/(range+eps), gs = gmin*s4 as [128,2] fb-columns."""
    m = np.stack([np.asarray(p, np.float32) for p in partials])  # [8,128,2,1024]
    m = m.reshape(N_CORES, 128, 2, 4, 256)
    gmin = m[:, :, 0].min(axis=(0, 1, 2))                        # [256]
    gmax = m[:, :, 1].max(axis=(0, 1, 2))
    qt = ((gmax - gmin) + np.float32(EPS_MINMAX)) * np.float32(0.25)
    s4 = (np.float32(1.0) / qt).astype(np.float32)
    gs = (gmin * s4).astype(np.float32)
    s4c = np.stack([s4[0:128], s4[128:256]], axis=1).astype(np.float32)
    gsc = np.stack([gs[0:128], gs[128:256]], axis=1).astype(np.float32)
    return s4c, gsc


def run_all(inputs, trace=False, tmpdirs=("/tmp/kan_trace1", "/tmp/kan_trace2")):
    if "nc1" not in _CACHE:
        _CACHE["nc1"] = _build_minmax()
        _CACHE["nc2"] = _build_main()
    nc1, nc2 = _CACHE["nc1"], _CACHE["nc2"]
    xs = _shard_x(inputs)
    w_sp, w_bs, bbc, csc = _host_prep(
        **{k: v for k, v in inputs.items() if k != "x"})
    ident = np.eye(128, dtype=NPBF16)

    kw1 = dict(trace=True, tmpdir=tmpdirs[0]) if trace else {}
    kw2 = dict(trace=True, tmpdir=tmpdirs[1]) if trace else {}
    res1 = run_bass_kernel_spmd(
        nc1, [{"x_sh": xs[c], "ident": ident} for c in range(N_CORES)],
        list(range(N_CORES)), **kw1)
    s4c, gsc = host_combine_minmax([res1.results[c]["mm_out"]
                                    for c in range(N_CORES)])
    in_maps = [{
        "xt_sh": res1.results[c]["xt_sh"],
        "w_sp": w_sp, "w_bs": w_bs, "bbc": bbc, "csc": csc,
        "s4c": s4c, "gsc": gsc,
    } for c in range(N_CORES)]
    res2 = run_bass_kernel_spmd(nc2, in_maps, list(range(N_CORES)), **kw2)
    arr = np.stack([res2.results[c]["out_sh"] for c in range(N_CORES)])
    out = arr.transpose(0, 3, 1, 2).reshape(N_TOTAL, OUT_F)
    out = np.ascontiguousarray(out).reshape(B, H, W, OUT_F).astype(np.float32)
    return out, res1, res2


def kernel(**inputs):
    out, _, _ = run_all(inputs)
    return out


# revision 25
# speedup vs baseline: 1.2114x; 1.2114x over previous
"""Trainium2 Bass kernel for nn_KANSplineLayer (KAN spline layer, 8-core SPMD).

Math rewrite (validated 2.9e-3 rel err vs reference on HW, tolerance 2e-2):
  reference: out = silu(BN_b(x @ Wb)) + BN_s(basis(minmax(x)) @ Ws.T)
  Spline is CPWL on t = 4*xn in [0,4] with kinks at {1,2,3}:
    out = silu(x@Wb + b_b) + t@Wt + relu(t-1)@H1 + relu(t-2)@H2 + relu(t-3)@H3 + C_s
  (contraction 4*256 instead of 9*256). All GEMM operands bf16 (fp32r streams
  at half PE rate; bf16 is full rate), fp32 PSUM accumulate.

Structure: TWO NEFF launches with a host-side 2KB-scale min/max combine
between them (the all-reduce across shards, host-mediated):
  NEFF-1: per-core DMA of its row shard (bf16) + running pairwise min/max on
      DVE -> per-core partial [128, 2, 256]. ~15us.
  host: reduce 8 x [128,2,256] partials -> global gmin/gmax per feature,
      s4 = 4/(range+eps), gs = gmin*s4 (f32, exact on bf16 values).
  NEFF-2: full compute with s4/gs as inputs. ~50us.
Measured alternatives that justify this: the on-device NCCL 2KB AllReduce
costs ~68us wall (fixed ~21+40us entry barrier + ~11us trigger delay + mesh
AR ~13-22us) and gates everything s4-dependent behind ~89us; a remote-DMA
XOR-butterfly (correct on HW) has ms-scale delivery latency in this axon
sandbox (host-emulated fabric), so neither beats two clean launches.

Other schedule decisions (from traces of earlier versions):
  * All GEMMs weight-stationary with TRANSPOSED output [out_f, rows]: one
    LDWEIGHTS per [128,128] weight block serves N=512-row matmuls
    (data-stationary pays LDW per matmul at ~275-300ns/MM: same row group ->
    no LDW/MM overlap). Out-feature biases (b_b inside silu, C_s in the
    epilogue) become per-partition scalars -- no rank-1 bias matmuls. Output
    is DMA'd transposed; the host untransposes (free).
  * Planes t,r1,r3 on DVE (tensor_scalar), r2 on ACT; ACT measured ~80-94
    G el/s so it also gets half the PSUM evacuations + silu, DVE the rest.
"""
import numpy as np
import ml_dtypes

import concourse.bacc as bacc
import concourse.bass as bass
import concourse.tile as tile
from concourse import mybir
from concourse.bass_utils import run_bass_kernel_spmd

# ---- problem constants (hardcoded; kernel.py must be self-contained) ----
IN_F, OUT_F = 256, 256
K_KNOTS = 9
EPS_MINMAX = 1e-7
EPS_BN = 1e-3
B, H, W = 32, 32, 32
N_TOTAL = B * H * W            # 32768 rows
N_CORES = 8
N_SHARD = N_TOTAL // N_CORES   # 4096 rows per core
R_TILES = N_SHARD // 128       # 32 row tiles per core
G_TILES = 4                    # row tiles per input DMA group
N_GROUPS = R_TILES // G_TILES  # 8 input DMA groups
CH = 512                       # chunk of rows (matmul moving N)
N_CHUNKS = N_SHARD // CH       # 8

F32 = mybir.dt.float32
BF16 = mybir.dt.bfloat16
NPBF16 = ml_dtypes.bfloat16


def _host_prep(base_weight, spline_weight, spline_scaler,
               bn_base_gamma, bn_base_beta, bn_base_mean, bn_base_var,
               bn_spline_gamma, bn_spline_beta, bn_spline_mean, bn_spline_var):
    """Fold BN + rewrite spline into relu-plane weights. All in float64."""
    f64 = np.float64
    w = np.asarray(spline_weight, f64) * np.asarray(spline_scaler, f64)[:, :, None]
    knots = np.linspace(-1.0, 1.0, K_KNOTS).astype(f64)
    jg = np.arange(5, dtype=f64) / 4.0
    tri = np.maximum(0.0, 1.0 - np.abs(jg[None, :] - knots[:, None]))   # [k, j]
    G = np.einsum('oik,kj->oij', w, tri)                                # [o,i,5]
    a_s = np.asarray(bn_spline_gamma, f64) / np.sqrt(np.asarray(bn_spline_var, f64) + EPS_BN)
    b_s = np.asarray(bn_spline_beta, f64) - a_s * np.asarray(bn_spline_mean, f64)
    G = G * a_s[:, None, None]
    W_t = (G[:, :, 1] - G[:, :, 0]).T                                   # [i,o]
    H1 = (G[:, :, 2] - 2 * G[:, :, 1] + G[:, :, 0]).T
    H2 = (G[:, :, 3] - 2 * G[:, :, 2] + G[:, :, 1]).T
    H3 = (G[:, :, 4] - 2 * G[:, :, 3] + G[:, :, 2]).T
    C_s = G[:, :, 0].sum(axis=1) + b_s                                  # [o]
    a_b = np.asarray(bn_base_gamma, f64) / np.sqrt(np.asarray(bn_base_var, f64) + EPS_BN)
    b_b = np.asarray(bn_base_beta, f64) - a_b * np.asarray(bn_base_mean, f64)
    Wb = np.asarray(base_weight, f64) * a_b[None, :]                    # [i,o]

    # weight blocks as lhsT [K=feat within fb, M=out within oh]
    planes = [W_t, H1, H2, H3]
    w_sp = np.empty((128, 4, 2, 2, 128), dtype=NPBF16)
    w_bs = np.empty((128, 2, 2, 128), dtype=NPBF16)
    for fb in range(2):
        for oh in range(2):
            for m, M in enumerate(planes):
                w_sp[:, m, fb, oh, :] = M[fb * 128:(fb + 1) * 128,
                                          oh * 128:(oh + 1) * 128].astype(NPBF16)
            w_bs[:, fb, oh, :] = Wb[fb * 128:(fb + 1) * 128,
                                    oh * 128:(oh + 1) * 128].astype(NPBF16)
    bbc = np.stack([b_b[0:128], b_b[128:256]], axis=1).astype(np.float32)  # [128,2]
    csc = np.stack([C_s[0:128], C_s[128:256]], axis=1).astype(np.float32)  # [128,2]
    return w_sp, w_bs, bbc, csc


def _build_minmax():
    """NEFF-1: per-shard running min/max + transpose x -> x^T writeback.

    DVE owns the min/max chain (the critical path); the otherwise-idle PE does
    the 64 [128,128] transposes and ACT + DVE split the PSUM evacuations.
    x^T goes back to DRAM so NEFF-2 starts matmuls immediately.
    """
    nc = bacc.Bacc(num_devices=N_CORES)
    x_sh = nc.declare_dram_parameter("x_sh", [N_SHARD, IN_F], BF16, isOutput=False)
    ident_d = nc.declare_dram_parameter("ident", [128, 128], BF16, isOutput=False)
    mm_out = nc.declare_dram_parameter("mm_out", [128, 2, 1024], BF16, isOutput=True)
    xt_d = nc.declare_dram_parameter("xt_sh", [2, 128, N_SHARD], BF16, isOutput=True)
    x_g = x_sh.rearrange("(g t p) f -> g p t f", g=N_GROUPS, t=G_TILES, p=128)
    xt_re = xt_d.rearrange("b p n -> p b n")

    from contextlib import ExitStack
    with tile.TileContext(nc) as tc, ExitStack() as es:
        cons = es.enter_context(tc.tile_pool(name="cons", bufs=1))
        stage = es.enter_context(tc.tile_pool(name="stage", bufs=2))
        xin_p = es.enter_context(tc.tile_pool(name="xin", bufs=3))
        xt_p = es.enter_context(tc.tile_pool(name="xtp", bufs=3))
        psT = es.enter_context(tc.tile_pool(name="psT", bufs=6, space="PSUM"))

        id_st = stage.tile([128, 128], BF16, tag="id_st")
        nc.sync.dma_start(out=id_st[:], in_=ident_d[:])
        ident = cons.tile([128, 128], BF16, tag="ident")
        nc.scalar.copy(out=ident[:], in_=id_st[:])

        accmin = cons.tile([128, G_TILES * IN_F], BF16, tag="accmin")
        accmax = cons.tile([128, G_TILES * IN_F], BF16, tag="accmax")
        for g in range(N_GROUPS):
            xin = xin_p.tile([128, G_TILES, IN_F], BF16, tag="xin", name=f"xin{g}")
            nc.sync.dma_start(out=xin[:], in_=x_g[g])
            v = xin[:].rearrange("p t f -> p (t f)")
            if g == 0:
                nc.vector.tensor_copy(out=accmin[:], in_=v)
                nc.vector.tensor_copy(out=accmax[:], in_=v)
            else:
                nc.vector.tensor_tensor(out=accmin[:], in0=accmin[:], in1=v,
                                        op=mybir.AluOpType.min)
                nc.vector.tensor_tensor(out=accmax[:], in0=accmax[:], in1=v,
                                        op=mybir.AluOpType.max)
            # transpose the group's 4 row tiles; evacs split DVE/ACT; DMA out
            xtg = xt_p.tile([128, 2, G_TILES * 128], BF16, tag="xtg",
                            name=f"xtg{g}")
            for t in range(G_TILES):
                for fb in range(2):
                    pst = psT.tile([128, 128], BF16, tag="pst")
                    nc.tensor.transpose(
                        pst[:], xin[:, t, fb * 128:(fb + 1) * 128], ident[:])
                    if (t * 2 + fb) % 8 < 3:
                        nc.vector.tensor_copy(
                            out=xtg[:, fb, t * 128:(t + 1) * 128], in_=pst[:])
                    else:
                        nc.scalar.copy(
                            out=xtg[:, fb, t * 128:(t + 1) * 128], in_=pst[:])
            for fb in range(2):
                nc.sync.dma_start(
                    out=xt_d[fb, :, g * 512:(g + 1) * 512], in_=xtg[:, fb, :])
        nc.sync.dma_start(out=mm_out[:, 0, :], in_=accmin[:])
        nc.sync.dma_start(out=mm_out[:, 1, :], in_=accmax[:])
    nc.compile()
    return nc


def _build_main():
    """NEFF-2: base path, planes, spline GEMMs, epilogue (reads x^T)."""
    nc = bacc.Bacc(num_devices=N_CORES)
    xt_d = nc.declare_dram_parameter("xt_sh", [2, 128, N_SHARD], BF16, isOutput=False)
    w_sp_d = nc.declare_dram_parameter("w_sp", [128, 4, 2, 2, 128], BF16, isOutput=False)
    w_bs_d = nc.declare_dram_parameter("w_bs", [128, 2, 2, 128], BF16, isOutput=False)
    bbc_d = nc.declare_dram_parameter("bbc", [128, 2], F32, isOutput=False)
    csc_d = nc.declare_dram_parameter("csc", [128, 2], F32, isOutput=False)
    s4c_d = nc.declare_dram_parameter("s4c", [128, 2], F32, isOutput=False)
    gsc_d = nc.declare_dram_parameter("gsc", [128, 2], F32, isOutput=False)
    out_sh = nc.declare_dram_parameter("out_sh", [2, 128, N_SHARD], F32, isOutput=True)
    xt_re = xt_d.rearrange("b p n -> p b n")

    from contextlib import ExitStack
    with tile.TileContext(nc) as tc, ExitStack() as es:
        cons = es.enter_context(tc.tile_pool(name="cons", bufs=1))
        stage = es.enter_context(tc.tile_pool(name="stage", bufs=2))
        planes_p = es.enter_context(tc.tile_pool(name="planes", bufs=2))
        outp = es.enter_context(tc.tile_pool(name="outp", bufs=4))
        psB = es.enter_context(tc.tile_pool(name="psB", bufs=2, space="PSUM"))
        psM = es.enter_context(tc.tile_pool(name="psM", bufs=2, space="PSUM"))

        xt = cons.tile([128, 2, N_SHARD], BF16, tag="xt")
        sb = cons.tile([128, 2, N_SHARD], BF16, tag="sb")

        def load(nm, shape, dram_ap, dt, eng):
            tmp = stage.tile(shape, dt, tag=f"st_{nm}", name=f"st_{nm}")
            nc.sync.dma_start(out=tmp[:], in_=dram_ap)
            dst = cons.tile(shape, dt, tag=nm, name=nm)
            if eng == "a":
                nc.scalar.copy(out=dst[:], in_=tmp[:])
            else:
                nc.vector.tensor_copy(out=dst[:], in_=tmp[:])
            return dst

        AF = mybir.ActivationFunctionType

        def emit_dma(c):
            csl = slice(c * CH, (c + 1) * CH)
            for fb in range(2):
                nc.sync.dma_start(out=xt[:, fb, csl], in_=xt_d[fb, :, csl])

        def emit_planes(c):
            csl = slice(c * CH, (c + 1) * CH)
            tpl = [[None, None] for _ in range(4)]
            for fb in range(2):
                t = planes_p.tile([128, CH], BF16, tag=f"t{fb}", name=f"t{fb}_{c}")
                nc.vector.tensor_scalar(
                    out=t[:], in0=xt[:, fb, csl],
                    scalar1=s4c[:, fb:fb + 1], scalar2=gsc[:, fb:fb + 1],
                    op0=mybir.AluOpType.mult, op1=mybir.AluOpType.subtract)
                tpl[0][fb] = t
                for m in (1, 2, 3):
                    rpl = planes_p.tile([128, CH], BF16, tag=f"r{m}{fb}",
                                        name=f"r{m}{fb}_{c}")
                    nc.vector.tensor_scalar(
                        out=rpl[:], in0=t[:], scalar1=float(m), scalar2=0.0,
                        op0=mybir.AluOpType.subtract, op1=mybir.AluOpType.max)
                    tpl[m][fb] = rpl
            return tpl

        def emit_base(c):
            csl = slice(c * CH, (c + 1) * CH)
            for oh in range(2):
                pb = psB.tile([128, CH], F32, tag=f"psb{oh}")
                nc.tensor.matmul(pb[:], w_bs[:, 0, oh, :], xt[:, 0, csl],
                                 start=True, stop=False, skip_group_check=True)
                nc.tensor.matmul(pb[:], w_bs[:, 1, oh, :], xt[:, 1, csl],
                                 start=False, stop=True, skip_group_check=True)
                nc.scalar.activation(
                    out=sb[:, oh, csl], in_=pb[:],
                    func=AF.Silu, bias=bbc[:, oh:oh + 1], scale=1.0)

        def emit_spline(c, tpl):
            csl = slice(c * CH, (c + 1) * CH)
            for oh in range(2):
                pm = psM.tile([128, CH], F32, tag=f"psm{oh}")
                first = True
                for m in range(4):
                    for fb in range(2):
                        nc.tensor.matmul(
                            pm[:], w_sp[:, m, fb, oh, :], tpl[m][fb][:],
                            start=first, stop=(m == 3 and fb == 1),
                            skip_group_check=True)
                        first = False
                o = outp.tile([128, CH], F32, tag=f"o{oh}", name=f"o{oh}_{c}")
                nc.vector.tensor_tensor(
                    out=o[:], in0=pm[:], in1=sb[:, oh, csl],
                    op=mybir.AluOpType.add)
                nc.vector.tensor_scalar(
                    out=o[:], in0=o[:], scalar1=csc[:, oh:oh + 1],
                    scalar2=None, op0=mybir.AluOpType.add)
                nc.sync.dma_start(out=out_sh[oh, :, csl], in_=o[:])

        w_bs = load("w_bs", [128, 2, 2, 128], w_bs_d[:], BF16, "a")
        bbc = load("bbc", [128, 2], bbc_d[:], F32, "a")
        s4c = load("s4c", [128, 2], s4c_d[:], F32, "v")
        gsc = load("gsc", [128, 2], gsc_d[:], F32, "v")
        csc = load("csc", [128, 2], csc_d[:], F32, "v")
        emit_dma(0)
        emit_dma(1)
        w_sp = load("w_sp", [128, 4, 2, 2, 128], w_sp_d[:], BF16, "a")

        tpl_prev = emit_planes(0)
        for c in range(N_CHUNKS):
            if c + 2 < N_CHUNKS:
                emit_dma(c + 2)
            emit_base(c)
            emit_spline(c, tpl_prev)
            if c + 1 < N_CHUNKS:
                tpl_prev = emit_planes(c + 1)
    nc.compile()
    return nc


_CACHE = {}


def _shard_x(inputs):
    xf = np.asarray(inputs["x"], np.float32).reshape(N_TOTAL, IN_F).astype(NPBF16)
    return [np.ascontiguousarray(xf[c * N_SHARD:(c + 1) * N_SHARD])
            for c in range(N_CORES)]


def host_combine_minmax(partials):
    """The cross-shard all-reduce of per-shard min/max partials (f32, exact on
    bf16 values), then s4 = 4/(range+eps), gs = gmin*s4 as [128,2] fb-columns."""
    m = np.stack([np.asarray(p, np.float32) for p in partials])  # [8,128,2,1024]
    m = m.reshape(N_CORES, 128, 2, 4, 256)
    gmin = m[:, :, 0].min(axis=(0, 1, 2))                        # [256]
    gmax = m[:, :, 1].max(axis=(0, 1, 2))
    qt = ((gmax - gmin) + np.float32(EPS_MINMAX)) * np.float32(0.25)
    s4 = (np.float32(1.0) / qt).astype(np.float32)
    gs = (gmin * s4).astype(np.float32)
    s4c = np.stack([s4[0:128], s4[128:256]], axis=1).astype(np.float32)
    gsc = np.stack([gs[0:128], gs[128:256]], axis=1).astype(np.float32)
    return s4c, gsc


def run_all(inputs, trace=False, tmpdirs=("/tmp/kan_trace1", "/tmp/kan_trace2")):
    if "nc1" not in _CACHE:
        _CACHE["nc1"] = _build_minmax()
        _CACHE["nc2"] = _build_main()
    nc1, nc2 = _CACHE["nc1"], _CACHE["nc2"]
    xs = _shard_x(inputs)
    w_sp, w_bs, bbc, csc = _host_prep(
        **{k: v for k, v in inputs.items() if k != "x"})
    ident = np.eye(128, dtype=NPBF16)

    kw1 = dict(trace=True, tmpdir=tmpdirs[0]) if trace else {}
    kw2 = dict(trace=True, tmpdir=tmpdirs[1]) if trace else {}
    res1 = run_bass_kernel_spmd(
        nc1, [{"x_sh": xs[c], "ident": ident} for c in range(N_CORES)],
        list(range(N_CORES)), **kw1)
    s4c, gsc = host_combine_minmax([res1.results[c]["mm_out"]
                                    for c in range(N_CORES)])
    in_maps = [{
        "xt_sh": res1.results[c]["xt_sh"],
        "w_sp": w_sp, "w_bs": w_bs, "bbc": bbc, "csc": csc,
        "s4c": s4c, "gsc": gsc,
    } for c in range(N_CORES)]
    res2 = run_bass_kernel_spmd(nc2, in_maps, list(range(N_CORES)), **kw2)
    arr = np.stack([res2.results[c]["out_sh"] for c in range(N_CORES)])
    out = arr.transpose(0, 3, 1, 2).reshape(N_TOTAL, OUT_F)
    out = np.ascontiguousarray(out).reshape(B, H, W, OUT_F).astype(np.float32)
    return out, res1, res2


def kernel(**inputs):
    out, _, _ = run_all(inputs)
    return out


# revision 26
# speedup vs baseline: 1.2510x; 1.0327x over previous
"""Trainium2 Bass kernel for nn_KANSplineLayer (KAN spline layer, 8-core SPMD).

Math rewrite (validated 2.9e-3 rel err vs reference on HW, tolerance 2e-2):
  reference: out = silu(BN_b(x @ Wb)) + BN_s(basis(minmax(x)) @ Ws.T)
  Spline is CPWL on t = 4*xn in [0,4] with kinks at {1,2,3}:
    out = silu(x@Wb + b_b) + t@Wt + relu(t-1)@H1 + relu(t-2)@H2 + relu(t-3)@H3 + C_s
  (contraction 4*256 instead of 9*256). All GEMM operands bf16 (fp32r streams
  at half PE rate; bf16 is full rate), fp32 PSUM accumulate.

Structure: TWO NEFF launches with a host-side 2KB-scale min/max combine
between them (the all-reduce across shards, host-mediated):
  NEFF-1: per-core DMA of its row shard (bf16) + running pairwise min/max on
      DVE -> per-core partial [128, 2, 256]. ~15us.
  host: reduce 8 x [128,2,256] partials -> global gmin/gmax per feature,
      s4 = 4/(range+eps), gs = gmin*s4 (f32, exact on bf16 values).
  NEFF-2: full compute with s4/gs as inputs. ~50us.
Measured alternatives that justify this: the on-device NCCL 2KB AllReduce
costs ~68us wall (fixed ~21+40us entry barrier + ~11us trigger delay + mesh
AR ~13-22us) and gates everything s4-dependent behind ~89us; a remote-DMA
XOR-butterfly (correct on HW) has ms-scale delivery latency in this axon
sandbox (host-emulated fabric), so neither beats two clean launches.

Other schedule decisions (from traces of earlier versions):
  * All GEMMs weight-stationary with TRANSPOSED output [out_f, rows]: one
    LDWEIGHTS per [128,128] weight block serves N=512-row matmuls
    (data-stationary pays LDW per matmul at ~275-300ns/MM: same row group ->
    no LDW/MM overlap). Out-feature biases (b_b inside silu, C_s in the
    epilogue) become per-partition scalars -- no rank-1 bias matmuls. Output
    is DMA'd transposed; the host untransposes (free).
  * Planes t,r1,r3 on DVE (tensor_scalar), r2 on ACT; ACT measured ~80-94
    G el/s so it also gets half the PSUM evacuations + silu, DVE the rest.
"""
import numpy as np
import ml_dtypes

import concourse.bacc as bacc
import concourse.bass as bass
import concourse.tile as tile
from concourse import mybir
from concourse.bass_utils import run_bass_kernel_spmd

# ---- problem constants (hardcoded; kernel.py must be self-contained) ----
IN_F, OUT_F = 256, 256
K_KNOTS = 9
EPS_MINMAX = 1e-7
EPS_BN = 1e-3
B, H, W = 32, 32, 32
N_TOTAL = B * H * W            # 32768 rows
N_CORES = 8
N_SHARD = N_TOTAL // N_CORES   # 4096 rows per core
R_TILES = N_SHARD // 128       # 32 row tiles per core
G_TILES = 4                    # row tiles per input DMA group
N_GROUPS = R_TILES // G_TILES  # 8 input DMA groups
CH = 512                       # chunk of rows (matmul moving N)
N_CHUNKS = N_SHARD // CH       # 8

F32 = mybir.dt.float32
BF16 = mybir.dt.bfloat16
NPBF16 = ml_dtypes.bfloat16


def _host_prep(base_weight, spline_weight, spline_scaler,
               bn_base_gamma, bn_base_beta, bn_base_mean, bn_base_var,
               bn_spline_gamma, bn_spline_beta, bn_spline_mean, bn_spline_var):
    """Fold BN + rewrite spline into relu-plane weights. All in float64."""
    f64 = np.float64
    w = np.asarray(spline_weight, f64) * np.asarray(spline_scaler, f64)[:, :, None]
    knots = np.linspace(-1.0, 1.0, K_KNOTS).astype(f64)
    jg = np.arange(5, dtype=f64) / 4.0
    tri = np.maximum(0.0, 1.0 - np.abs(jg[None, :] - knots[:, None]))   # [k, j]
    G = np.einsum('oik,kj->oij', w, tri)                                # [o,i,5]
    a_s = np.asarray(bn_spline_gamma, f64) / np.sqrt(np.asarray(bn_spline_var, f64) + EPS_BN)
    b_s = np.asarray(bn_spline_beta, f64) - a_s * np.asarray(bn_spline_mean, f64)
    G = G * a_s[:, None, None]
    W_t = (G[:, :, 1] - G[:, :, 0]).T                                   # [i,o]
    H1 = (G[:, :, 2] - 2 * G[:, :, 1] + G[:, :, 0]).T
    H2 = (G[:, :, 3] - 2 * G[:, :, 2] + G[:, :, 1]).T
    H3 = (G[:, :, 4] - 2 * G[:, :, 3] + G[:, :, 2]).T
    C_s = G[:, :, 0].sum(axis=1) + b_s                                  # [o]
    a_b = np.asarray(bn_base_gamma, f64) / np.sqrt(np.asarray(bn_base_var, f64) + EPS_BN)
    b_b = np.asarray(bn_base_beta, f64) - a_b * np.asarray(bn_base_mean, f64)
    Wb = np.asarray(base_weight, f64) * a_b[None, :]                    # [i,o]

    # weight blocks as lhsT [K=feat within fb, M=out within oh]
    planes = [W_t, H1, H2, H3]
    w_sp = np.empty((128, 4, 2, 2, 128), dtype=NPBF16)
    w_bs = np.empty((128, 2, 2, 128), dtype=NPBF16)
    for fb in range(2):
        for oh in range(2):
            for m, M in enumerate(planes):
                w_sp[:, m, fb, oh, :] = M[fb * 128:(fb + 1) * 128,
                                          oh * 128:(oh + 1) * 128].astype(NPBF16)
            w_bs[:, fb, oh, :] = Wb[fb * 128:(fb + 1) * 128,
                                    oh * 128:(oh + 1) * 128].astype(NPBF16)
    bbc = np.stack([b_b[0:128], b_b[128:256]], axis=1).astype(np.float32)  # [128,2]
    csc = np.stack([C_s[0:128], C_s[128:256]], axis=1).astype(np.float32)  # [128,2]
    return w_sp, w_bs, bbc, csc


def _build_minmax():
    """NEFF-1: per-shard running min/max -> [128, 2, 1024] raw accumulators
    (host does the final cross-partial/cross-core reduce)."""
    nc = bacc.Bacc(num_devices=N_CORES)
    x_sh = nc.declare_dram_parameter("x_sh", [N_SHARD, IN_F], BF16, isOutput=False)
    mm_out = nc.declare_dram_parameter("mm_out", [128, 2, 1024], BF16, isOutput=True)
    x_g = x_sh.rearrange("(g t p) f -> g p t f", g=N_GROUPS, t=G_TILES, p=128)

    from contextlib import ExitStack
    with tile.TileContext(nc) as tc, ExitStack() as es:
        cons = es.enter_context(tc.tile_pool(name="cons", bufs=1))
        xin_p = es.enter_context(tc.tile_pool(name="xin", bufs=3))
        accmin = cons.tile([128, G_TILES * IN_F], BF16, tag="accmin")
        accmax = cons.tile([128, G_TILES * IN_F], BF16, tag="accmax")
        for g in range(N_GROUPS):
            xin = xin_p.tile([128, G_TILES, IN_F], BF16, tag="xin", name=f"xin{g}")
            nc.sync.dma_start(out=xin[:], in_=x_g[g])
            v = xin[:].rearrange("p t f -> p (t f)")
            if g == 0:
                nc.vector.tensor_copy(out=accmin[:], in_=v)
                nc.vector.tensor_copy(out=accmax[:], in_=v)
            else:
                nc.vector.tensor_tensor(out=accmin[:], in0=accmin[:], in1=v,
                                        op=mybir.AluOpType.min)
                nc.vector.tensor_tensor(out=accmax[:], in0=accmax[:], in1=v,
                                        op=mybir.AluOpType.max)
        nc.sync.dma_start(out=mm_out[:, 0, :], in_=accmin[:])
        nc.sync.dma_start(out=mm_out[:, 1, :], in_=accmax[:])
    nc.compile()
    return nc


def _build_main():
    """NEFF-2: transposes, base path, planes, spline GEMMs, epilogue.

    Chunk-major pipeline (one 512-row chunk = one DMA group of 4 row tiles):
    transposes -> evacuations -> planes (DVE) -> base matmuls -> spline
    matmuls -> epilogue -> out-DMA. Real matmuls start early and stay dense,
    keeping the PE HAM clock warm."""
    nc = bacc.Bacc(num_devices=N_CORES)
    x_sh = nc.declare_dram_parameter("x_sh", [N_SHARD, IN_F], BF16, isOutput=False)
    w_sp_d = nc.declare_dram_parameter("w_sp", [128, 4, 2, 2, 128], BF16, isOutput=False)
    w_bs_d = nc.declare_dram_parameter("w_bs", [128, 2, 2, 128], BF16, isOutput=False)
    bbc_d = nc.declare_dram_parameter("bbc", [128, 2], F32, isOutput=False)
    csc_d = nc.declare_dram_parameter("csc", [128, 2], F32, isOutput=False)
    s4c_d = nc.declare_dram_parameter("s4c", [128, 2], F32, isOutput=False)
    gsc_d = nc.declare_dram_parameter("gsc", [128, 2], F32, isOutput=False)
    ident_d = nc.declare_dram_parameter("ident", [128, 128], BF16, isOutput=False)
    out_sh = nc.declare_dram_parameter("out_sh", [2, 128, N_SHARD], F32, isOutput=True)

    x_g = x_sh.rearrange("(g t p) f -> g p t f", g=N_GROUPS, t=G_TILES, p=128)

    from contextlib import ExitStack
    with tile.TileContext(nc) as tc, ExitStack() as es:
        cons = es.enter_context(tc.tile_pool(name="cons", bufs=1))
        stage = es.enter_context(tc.tile_pool(name="stage", bufs=2))
        xin_p = es.enter_context(tc.tile_pool(name="xin", bufs=N_GROUPS))
        planes_p = es.enter_context(tc.tile_pool(name="planes", bufs=2))
        outp = es.enter_context(tc.tile_pool(name="outp", bufs=4))
        psT = es.enter_context(tc.tile_pool(name="psT", bufs=4, space="PSUM"))
        psB = es.enter_context(tc.tile_pool(name="psB", bufs=1, space="PSUM"))
        psM = es.enter_context(tc.tile_pool(name="psM", bufs=1, space="PSUM"))

        # identity for PE transposes (tiny, first in the DMA queue)
        id_st = stage.tile([128, 128], BF16, tag="id_st")
        nc.sync.dma_start(out=id_st[:], in_=ident_d[:])
        ident = cons.tile([128, 128], BF16, tag="ident")
        nc.scalar.copy(out=ident[:], in_=id_st[:])

        xt = cons.tile([128, 2, N_SHARD], BF16, tag="xt")
        sb = cons.tile([128, 2, N_SHARD], BF16, tag="sb")

        def load(nm, shape, dram_ap, dt, eng):
            tmp = stage.tile(shape, dt, tag=f"st_{nm}", name=f"st_{nm}")
            nc.sync.dma_start(out=tmp[:], in_=dram_ap)
            dst = cons.tile(shape, dt, tag=nm, name=nm)
            if eng == "a":
                nc.scalar.copy(out=dst[:], in_=tmp[:])
            else:
                nc.vector.tensor_copy(out=dst[:], in_=tmp[:])
            return dst

        AF = mybir.ActivationFunctionType
        xins = []

        def emit_dma(g):
            xin = xin_p.tile([128, G_TILES, IN_F], BF16, tag="xin", name=f"xin{g}")
            nc.sync.dma_start(out=xin[:], in_=x_g[g])
            xins.append(xin)

        def emit_transposes(c):
            xin = xins[c]
            for t in range(G_TILES):
                r = c * G_TILES + t
                for fb in range(2):
                    pst = psT.tile([128, 128], BF16, tag="pst")
                    nc.tensor.transpose(
                        pst[:], xin[:, t, fb * 128:(fb + 1) * 128], ident[:])
                    if fb == 0:
                        nc.vector.tensor_copy(
                            out=xt[:, fb, r * 128:(r + 1) * 128], in_=pst[:])
                    else:
                        nc.scalar.copy(
                            out=xt[:, fb, r * 128:(r + 1) * 128], in_=pst[:])

        def emit_planes(c):
            csl = slice(c * CH, (c + 1) * CH)
            tpl = [[None, None] for _ in range(4)]
            for fb in range(2):
                t = planes_p.tile([128, CH], BF16, tag=f"t{fb}", name=f"t{fb}_{c}")
                nc.vector.tensor_scalar(
                    out=t[:], in0=xt[:, fb, csl],
                    scalar1=s4c[:, fb:fb + 1], scalar2=gsc[:, fb:fb + 1],
                    op0=mybir.AluOpType.mult, op1=mybir.AluOpType.subtract)
                tpl[0][fb] = t
                for m in (1, 2, 3):
                    rpl = planes_p.tile([128, CH], BF16, tag=f"r{m}{fb}",
                                        name=f"r{m}{fb}_{c}")
                    nc.vector.tensor_scalar(
                        out=rpl[:], in0=t[:], scalar1=float(m), scalar2=0.0,
                        op0=mybir.AluOpType.subtract, op1=mybir.AluOpType.max)
                    tpl[m][fb] = rpl
            return tpl

        def emit_base(c):
            csl = slice(c * CH, (c + 1) * CH)
            for oh in range(2):
                pb = psB.tile([128, CH], F32, tag=f"psb{oh}")
                nc.tensor.matmul(pb[:], w_bs[:, 0, oh, :], xt[:, 0, csl],
                                 start=True, stop=False, skip_group_check=True)
                nc.tensor.matmul(pb[:], w_bs[:, 1, oh, :], xt[:, 1, csl],
                                 start=False, stop=True, skip_group_check=True)
                nc.scalar.activation(
                    out=sb[:, oh, csl], in_=pb[:],
                    func=AF.Silu, bias=bbc[:, oh:oh + 1], scale=1.0)

        def emit_spline(c, tpl):
            csl = slice(c * CH, (c + 1) * CH)
            for oh in range(2):
                pm = psM.tile([128, CH], F32, tag=f"psm{oh}")
                first = True
                for m in range(4):
                    for fb in range(2):
                        nc.tensor.matmul(
                            pm[:], w_sp[:, m, fb, oh, :], tpl[m][fb][:],
                            start=first, stop=(m == 3 and fb == 1),
                            skip_group_check=True)
                        first = False
                o = outp.tile([128, CH], F32, tag=f"o{oh}", name=f"o{oh}_{c}")
                nc.vector.tensor_tensor(
                    out=o[:], in0=pm[:], in1=sb[:, oh, csl],
                    op=mybir.AluOpType.add)
                nc.vector.tensor_scalar(
                    out=o[:], in0=o[:], scalar1=csc[:, oh:oh + 1],
                    scalar2=None, op0=mybir.AluOpType.add)
                nc.sync.dma_start(out=out_sh[oh, :, csl], in_=o[:])

        # small params first so w_bs/scalars don't queue behind w_sp's DMA
        w_bs = load("w_bs", [128, 2, 2, 128], w_bs_d[:], BF16, "a")
        bbc = load("bbc", [128, 2], bbc_d[:], F32, "a")
        s4c = load("s4c", [128, 2], s4c_d[:], F32, "v")
        gsc = load("gsc", [128, 2], gsc_d[:], F32, "v")
        csc = load("csc", [128, 2], csc_d[:], F32, "v")
        emit_dma(0)
        emit_dma(1)
        w_sp = load("w_sp", [128, 4, 2, 2, 128], w_sp_d[:], BF16, "a")

        emit_transposes(0)
        tpl_prev = emit_planes(0)
        for c in range(N_CHUNKS):
            if c + 2 < N_CHUNKS:
                emit_dma(c + 2)
            if c + 1 < N_CHUNKS:
                emit_transposes(c + 1)
            emit_base(c)
            emit_spline(c, tpl_prev)
            if c + 1 < N_CHUNKS:
                tpl_prev = emit_planes(c + 1)
    nc.compile()
    return nc


_CACHE = {}


def _shard_x(inputs):
    xf = np.asarray(inputs["x"], np.float32).reshape(N_TOTAL, IN_F).astype(NPBF16)
    return [np.ascontiguousarray(xf[c * N_SHARD:(c + 1) * N_SHARD])
            for c in range(N_CORES)]


def host_combine_minmax(partials):
    """The cross-shard all-reduce of per-shard min/max partials (f32, exact on
    bf16 values), then s4 = 4/(range+eps), gs = gmin*s4 as [128,2] fb-columns."""
    m = np.stack([np.asarray(p, np.float32) for p in partials])  # [8,128,2,1024]
    m = m.reshape(N_CORES, 128, 2, 4, 256)
    gmin = m[:, :, 0].min(axis=(0, 1, 2))                        # [256]
    gmax = m[:, :, 1].max(axis=(0, 1, 2))
    qt = ((gmax - gmin) + np.float32(EPS_MINMAX)) * np.float32(0.25)
    s4 = (np.float32(1.0) / qt).astype(np.float32)
    gs = (gmin * s4).astype(np.float32)
    s4c = np.stack([s4[0:128], s4[128:256]], axis=1).astype(np.float32)
    gsc = np.stack([gs[0:128], gs[128:256]], axis=1).astype(np.float32)
    return s4c, gsc


def run_all(inputs, trace=False, tmpdirs=("/tmp/kan_trace1", "/tmp/kan_trace2")):
    if "nc1" not in _CACHE:
        _CACHE["nc1"] = _build_minmax()
        _CACHE["nc2"] = _build_main()
    nc1, nc2 = _CACHE["nc1"], _CACHE["nc2"]
    xs = _shard_x(inputs)
    w_sp, w_bs, bbc, csc = _host_prep(
        **{k: v for k, v in inputs.items() if k != "x"})
    ident = np.eye(128, dtype=NPBF16)

    kw1 = dict(trace=True, tmpdir=tmpdirs[0]) if trace else {}
    kw2 = dict(trace=True, tmpdir=tmpdirs[1]) if trace else {}
    res1 = run_bass_kernel_spmd(
        nc1, [{"x_sh": xs[c], "ident": ident} for c in range(N_CORES)],
        list(range(N_CORES)), **kw1)
    s4c, gsc = host_combine_minmax([res1.results[c]["mm_out"]
                                    for c in range(N_CORES)])
    in_maps = [{
        "x_sh": xs[c], "w_sp": w_sp, "w_bs": w_bs, "bbc": bbc, "csc": csc,
        "s4c": s4c, "gsc": gsc, "ident": ident,
    } for c in range(N_CORES)]
    res2 = run_bass_kernel_spmd(nc2, in_maps, list(range(N_CORES)), **kw2)
    arr = np.stack([res2.results[c]["out_sh"] for c in range(N_CORES)])
    out = arr.transpose(0, 3, 1, 2).reshape(N_TOTAL, OUT_F)
    out = np.ascontiguousarray(out).reshape(B, H, W, OUT_F).astype(np.float32)
    return out, res1, res2


def kernel(**inputs):
    out, _, _ = run_all(inputs)
    return out
